# revision 11
# baseline (speedup 1.0000x reference)
"""Multi-head self-attention (B=4,S=2048,D=1024,H=16,DH=64, causal) on 8 trn2 cores.

Sharding: core c -> batch b=c//2, head-group g=c%2 (8 heads each).

v3 (from 458us v1, 310us v2):
 - ScalarE runs ONLY the softmax exps (one fused ACT per head-pair per key
   block, diagonal blocks trimmed to live columns).  Scale/bq folded into the
   Q-projection (host-prescaled weights + per-partition scalar on the PSUM->
   SBUF copy); bk dropped exactly via softmax shift invariance; bv/bp folded
   into a host-side constant (attn rows sum to 1).
 - Attention phase is exp-paced; PE head-of-line stalls at the AV matmuls are
   filled by interleaving the previous block's out-projection and the NEXT
   s-block's QKV chains (2 filler matmuls per key block) into the t loop.
 - Score-pair matmuls adjacent -> row-group (64/64) concurrent on the array.
 - Renorm: one [65,512] PSUM->SBUF copy frees the accumulator bank early,
   then reciprocal_approx_fast (SBUF-only!) + gpsimd partition_broadcast.

K-projection quirk (reference views k as (B,S,DH,H)): head h uses Wk rows
[dh*16+h for dh in range(64)] -- handled by host-side row gather.
"""
import numpy as np

import concourse.mybir as mybir
import concourse.tile as tile
from concourse import bacc
from concourse.bass_utils import run_bass_kernel_spmd

F32 = mybir.dt.float32
BF16 = mybir.dt.bfloat16
AF = mybir.ActivationFunctionType
MUL = mybir.AluOpType.mult

B, S, D, H, DH = 4, 2048, 1024, 16, 64
FG = 512          # features per head-group (8 heads * 64)
N_CORES = 8
SCALE = 0.125     # 1/sqrt(64)

_NC = None


def _build():
    nc = bacc.Bacc("TRN2", target_bir_lowering=False, debug=False,
                   num_devices=N_CORES, enable_asserts=False)
    xT_d = nc.dram_tensor("xT", [D, S], F32, kind="ExternalInput").ap()
    wqT_d = nc.dram_tensor("wqT", [D, FG], F32, kind="ExternalInput").ap()
    wkT_d = nc.dram_tensor("wkT", [D, FG], F32, kind="ExternalInput").ap()
    wvT_d = nc.dram_tensor("wvT", [D, FG], F32, kind="ExternalInput").ap()
    wpT_d = nc.dram_tensor("wpT", [FG, D], F32, kind="ExternalInput").ap()
    bqs_d = nc.dram_tensor("bqs", [128, 4], F32, kind="ExternalInput").ap()
    msk_d = nc.dram_tensor("msk", [128, 2, 128], F32, kind="ExternalInput").ap()
    out_d = nc.dram_tensor("outT", [D, S], F32, kind="ExternalOutput").ap()

    with tile.TileContext(nc) as tc:
        with tc.tile_pool(name="persist", bufs=1) as pp, \
             tc.tile_pool(name="xin", bufs=2) as xp, \
             tc.tile_pool(name="etile", bufs=4) as ep, \
             tc.tile_pool(name="small", bufs=8) as sp, \
             tc.tile_pool(name="avd", bufs=3) as ap_, \
             tc.tile_pool(name="outtile", bufs=3) as op, \
             tc.tile_pool(name="psprs", bufs=2, space="PSUM") as ps_s, \
             tc.tile_pool(name="psoth", bufs=2, space="PSUM") as ps_o:

            # ---- persistent SBUF tensors ----
            wq = pp.tile([128, 8, FG], BF16)   # [dp, do, f]
            wk = pp.tile([128, 8, FG], BF16)
            wv = pp.tile([128, 8, FG], BF16)
            wp = pp.tile([128, 4, D], BF16)    # [cp, co, j]
            qt = pp.tile([128, 4, S], BF16)    # [fp, fo, s]
            kt = pp.tile([128, 4, S], BF16)
            va = pp.tile([128, 16, 8, DH + 1], BF16)  # [skp, sko, h, dh|1]
            on_ = pp.tile([128, 4, S], BF16)   # renormed out^T  [cp, co, s]
            msk = pp.tile([128, 2, 128], BF16)
            bqs = pp.tile([128, 4], F32)

            # casting DMAs must use the gpsimd queue; stagger weight loads so
            # the first matmuls can start as early as possible
            nc.gpsimd.dma_start(wq[:], wqT_d.rearrange("(do dp) f -> dp do f", dp=128))
            nc.gpsimd.dma_start(wk[:], wkT_d.rearrange("(do dp) f -> dp do f", dp=128))
            nc.sync.dma_start(bqs[:], bqs_d[:])
            nc.vector.memset(va[:, :, :, DH:DH + 1], 1.0)

            xT_r = xT_d.rearrange("(do dp) s -> dp do s", dp=128)

            def qkv_steps(sb, xblk, steps):
                # one thunk per matmul; PSUM->SBUF copy rides on the last
                for wt, dst, bias in ((wq, qt, bqs), (wk, kt, None)):
                    for ft in range(4):
                        st = {}
                        for do in range(8):
                            def th(wt=wt, dst=dst, bias=bias, ft=ft, do=do,
                                   st=st, sb=sb, xblk=xblk):
                                if do == 0:
                                    st['ps'] = ps_o.tile([128, 512], F32, name="psc",
                                                         space="PSUM", tag="mm")
                                nc.tensor.matmul(
                                    st['ps'][:], wt[:, do, ft * 128:(ft + 1) * 128],
                                    xblk[:, do, :], start=(do == 0), stop=(do == 7))
                                if do == 7:
                                    d = dst[:, ft, sb * 512:(sb + 1) * 512]
                                    if bias is not None:
                                        nc.vector.tensor_scalar_add(
                                            d, st['ps'][:], bias[:, ft:ft + 1])
                                    else:
                                        nc.vector.tensor_copy(d, st['ps'][:])
                            steps.append(th)
                for stt in range(4):
                    st = {}
                    for do in range(8):
                        def th(stt=stt, do=do, st=st, sb=sb, xblk=xblk):
                            if do == 0:
                                st['ps'] = ps_o.tile([128, 512], F32, name="psv",
                                                     space="PSUM", tag="mm")
                            nc.tensor.matmul(
                                st['ps'][:], xblk[:, do, stt * 128:(stt + 1) * 128],
                                wv[:, do, :], start=(do == 0), stop=(do == 7))
                            if do == 7:
                                nc.vector.tensor_copy(
                                    va[:, sb * 4 + stt, :, :DH],
                                    st['ps'][:].rearrange("p (h d) -> p h d", h=8))
                        steps.append(th)

            def proj_steps(bb, jts, steps):
                for jt in jts:
                    st = {}
                    for co in range(4):
                        def th(jt=jt, co=co, st=st, bb=bb):
                            if co == 0:
                                st['ps'] = ps_o.tile([128, 512], F32, name="psj",
                                                     space="PSUM", tag="mm")
                            nc.tensor.matmul(
                                st['ps'][:], wp[:, co, jt * 128:(jt + 1) * 128],
                                on_[:, co, bb * 512:(bb + 1) * 512],
                                start=(co == 0), stop=(co == 3))
                            if co == 3:
                                osb = op.tile([128, 512], F32, tag="o")
                                nc.vector.tensor_copy(osb[:], st['ps'][:])
                                nc.sync.dma_start(
                                    out_d[jt * 128:(jt + 1) * 128,
                                          bb * 512:(bb + 1) * 512], osb[:])
                        steps.append(th)

            # ---- QKV for s-block 0 runs upfront (nothing to overlap yet) ----
            xblk0 = xp.tile([128, 8, 512], BF16, tag="x")
            nc.gpsimd.dma_start(xblk0[:], xT_r[:, :, 0:512])
            nc.gpsimd.dma_start(msk[:], msk_d[:])
            nc.gpsimd.dma_start(wv[:], wvT_d.rearrange("(do dp) f -> dp do f", dp=128))
            nc.gpsimd.dma_start(wp[:], wpT_d.rearrange("(co cp) j -> cp co j", cp=128))
            s0 = []
            qkv_steps(0, xblk0, s0)
            for th in s0:
                th()

            for b in range(4):
                steps = []
                if b >= 1:
                    proj_steps(b - 1, range(8), steps)
                if b < 3:
                    xblkn = xp.tile([128, 8, 512], BF16, tag="x")
                    nc.gpsimd.dma_start(
                        xblkn[:], xT_r[:, :, (b + 1) * 512:(b + 2) * 512])
                    qkv_steps(b + 1, xblkn, steps)
                it = iter(steps)

                nt = 4 * b + 4
                for p in range(4):  # head pairs (2p, 2p+1)
                    ot0 = ps_o.tile([DH + 1, 512], F32, space="PSUM", tag="ot")
                    ot1 = ps_o.tile([DH + 1, 512], F32, space="PSUM", tag="ot")
                    for t in range(nt):
                        m = t - 4 * b          # >= 0 on diagonal blocks
                        c0 = 128 * m if m > 0 else 0
                        ksl = slice(t * 128, (t + 1) * 128)
                        qsl = slice(b * 512 + c0, (b + 1) * 512)
                        spr = ps_s.tile([128, 2, 512], F32, space="PSUM", tag="s")
                        nc.tensor.matmul(spr[:, 0, c0:], kt[0:64, p, ksl],
                                         qt[0:64, p, qsl], start=True, stop=True)
                        nc.tensor.matmul(spr[:, 1, c0:], kt[64:128, p, ksl],
                                         qt[64:128, p, qsl], start=True, stop=True)
                        e = ep.tile([128, 2, 512], BF16, tag="e")
                        nc.scalar.activation(e[:, :, c0:], spr[:, :, c0:], AF.Exp)
                        if m >= 0:  # diagonal block: causal mask on the 128-strip
                            nc.vector.tensor_tensor(e[:, :, c0:c0 + 128],
                                                    e[:, :, c0:c0 + 128],
                                                    msk[:], MUL)
                        for _ in range(2):  # PE filler while ScalarE runs exp
                            th = next(it, None)
                            if th is not None:
                                th()
                        nc.tensor.matmul(ot0[:, c0:], va[:, t, 2 * p, :],
                                         e[:, 0, c0:],
                                         start=(t == 0), stop=(t == nt - 1),
                                         skip_group_check=True)
                        nc.tensor.matmul(ot1[:, c0:], va[:, t, 2 * p + 1, :],
                                         e[:, 1, c0:],
                                         start=(t == 0), stop=(t == nt - 1),
                                         skip_group_check=True)
                    for h, otp in ((2 * p, ot0), (2 * p + 1, ot1)):
                        den = sp.tile([1, 512], F32, tag="den")
                        nc.vector.tensor_copy(den[:], otp[DH:DH + 1, :])
                        rec = sp.tile([1, 512], F32, tag="rec")
                        nc.vector.reciprocal_approx_fast(rec[:], den[:])
                        rb = sp.tile([DH, 512], F32, tag="rb")
                        nc.gpsimd.partition_broadcast(rb[:], rec[:])
                        r0 = 64 * (h % 2)
                        dst = on_[r0:r0 + 64, p, b * 512:(b + 1) * 512]
                        nc.vector.tensor_tensor(dst, otp[0:DH, :], rb[:], MUL)
                for th in it:  # drain leftover filler
                    th()
            s3 = []
            proj_steps(3, range(8), s3)
            for th in s3:
                th()

    nc.compile()
    return nc


def kernel(x, Wq, bq, Wk, bk, Wv, bv, Wp, bp):
    global _NC
    if _NC is None:
        _NC = _build()

    x = np.asarray(x, np.float32)
    Wq, bq = np.asarray(Wq, np.float32), np.asarray(bq, np.float32)
    Wk, bk = np.asarray(Wk, np.float32), np.asarray(bk, np.float32)
    Wv, bv = np.asarray(Wv, np.float32), np.asarray(bv, np.float32)
    Wp, bp = np.asarray(Wp, np.float32), np.asarray(bp, np.float32)

    # diagonal-strip causal mask, duplicated for the two heads of a pair
    i = np.arange(128)[:, None]
    j = np.arange(128)[None, :]
    mstrip = (i <= j).astype(np.float32)            # [128, 128]
    msk = np.broadcast_to(mstrip[:, None, :], (128, 2, 128)).copy()

    # host-folded constant: attn rows sum to 1 -> out += bv, then @Wp.T
    host_bias = Wp @ bv + bp                        # [D]

    in_maps = []
    for c in range(N_CORES):
        b, g = c // 2, c % 2
        hs = range(8 * g, 8 * g + 8)
        kidx = np.array([dh * 16 + h for h in hs for dh in range(DH)])
        fsl = slice(FG * g, FG * (g + 1))
        in_maps.append({
            "xT": np.ascontiguousarray(x[b].T),
            "wqT": np.ascontiguousarray((SCALE * Wq[fsl]).T),
            "wkT": np.ascontiguousarray(Wk[kidx].T),
            "wvT": np.ascontiguousarray(Wv[fsl].T),
            "wpT": np.ascontiguousarray(Wp[:, fsl].T),
            "bqs": np.ascontiguousarray((SCALE * bq[fsl]).reshape(4, 128).T),
            "msk": msk,
        })

    res = run_bass_kernel_spmd(_NC, in_maps, core_ids=list(range(N_CORES)))
    out = np.empty((B, S, D), np.float32)
    for b in range(B):
        acc = res.results[2 * b]["outT"] + res.results[2 * b + 1]["outT"]
        out[b] = acc.T + host_bias
    return out


# revision 12
# speedup vs baseline: 1.0051x; 1.0051x over previous
"""Multi-head self-attention (B=4,S=2048,D=1024,H=16,DH=64, causal) on 8 trn2 cores.

Sharding: core c -> batch b=c//2, head-group g=c%2 (8 heads each).

v3 (from 458us v1, 310us v2):
 - ScalarE runs ONLY the softmax exps (one fused ACT per head-pair per key
   block, diagonal blocks trimmed to live columns).  Scale/bq folded into the
   Q-projection (host-prescaled weights + per-partition scalar on the PSUM->
   SBUF copy); bk dropped exactly via softmax shift invariance; bv/bp folded
   into a host-side constant (attn rows sum to 1).
 - Attention phase is exp-paced; PE head-of-line stalls at the AV matmuls are
   filled by interleaving the previous block's out-projection and the NEXT
   s-block's QKV chains (2 filler matmuls per key block) into the t loop.
 - Score-pair matmuls adjacent -> row-group (64/64) concurrent on the array.
 - Renorm: one [65,512] PSUM->SBUF copy frees the accumulator bank early,
   then reciprocal_approx_fast (SBUF-only!) + gpsimd partition_broadcast.

K-projection quirk (reference views k as (B,S,DH,H)): head h uses Wk rows
[dh*16+h for dh in range(64)] -- handled by host-side row gather.
"""
import numpy as np

import concourse.mybir as mybir
import concourse.tile as tile
from concourse import bacc
from concourse.bass_utils import run_bass_kernel_spmd

F32 = mybir.dt.float32
BF16 = mybir.dt.bfloat16
AF = mybir.ActivationFunctionType
MUL = mybir.AluOpType.mult

B, S, D, H, DH = 4, 2048, 1024, 16, 64
FG = 512          # features per head-group (8 heads * 64)
N_CORES = 8
SCALE = 0.125     # 1/sqrt(64)

_NC = None


def _build():
    nc = bacc.Bacc("TRN2", target_bir_lowering=False, debug=False,
                   num_devices=N_CORES, enable_asserts=False)
    xT_d = nc.dram_tensor("xT", [D, S], F32, kind="ExternalInput").ap()
    wqT_d = nc.dram_tensor("wqT", [D, FG], F32, kind="ExternalInput").ap()
    wkT_d = nc.dram_tensor("wkT", [D, FG], F32, kind="ExternalInput").ap()
    wvT_d = nc.dram_tensor("wvT", [D, FG], F32, kind="ExternalInput").ap()
    wpT_d = nc.dram_tensor("wpT", [FG, D], F32, kind="ExternalInput").ap()
    bqs_d = nc.dram_tensor("bqs", [128, 4], F32, kind="ExternalInput").ap()
    msk_d = nc.dram_tensor("msk", [128, 2, 128], F32, kind="ExternalInput").ap()
    out_d = nc.dram_tensor("outT", [D, S], F32, kind="ExternalOutput").ap()

    with tile.TileContext(nc) as tc:
        with tc.tile_pool(name="persist", bufs=1) as pp, \
             tc.tile_pool(name="xin", bufs=2) as xp, \
             tc.tile_pool(name="etile", bufs=4) as ep, \
             tc.tile_pool(name="small", bufs=8) as sp, \
             tc.tile_pool(name="avd", bufs=3) as ap_, \
             tc.tile_pool(name="outtile", bufs=3) as op, \
             tc.tile_pool(name="psprs", bufs=2, space="PSUM") as ps_s, \
             tc.tile_pool(name="psoth", bufs=2, space="PSUM") as ps_o:

            # ---- persistent SBUF tensors ----
            wq = pp.tile([128, 8, FG], BF16)   # [dp, do, f]
            wk = pp.tile([128, 8, FG], BF16)
            wv = pp.tile([128, 8, FG], BF16)
            wp = pp.tile([128, 4, D], BF16)    # [cp, co, j]
            qt = pp.tile([128, 4, S], BF16)    # [fp, fo, s]
            kt = pp.tile([128, 4, S], BF16)
            va = pp.tile([128, 16, 8, DH + 1], BF16)  # [skp, sko, h, dh|1]
            on_ = pp.tile([128, 4, S], BF16)   # renormed out^T  [cp, co, s]
            msk = pp.tile([128, 2, 128], BF16)
            bqs = pp.tile([128, 4], F32)

            # casting DMAs must use the gpsimd queue; stagger weight loads so
            # the first matmuls can start as early as possible
            nc.gpsimd.dma_start(wq[:], wqT_d.rearrange("(do dp) f -> dp do f", dp=128))
            nc.gpsimd.dma_start(wk[:], wkT_d.rearrange("(do dp) f -> dp do f", dp=128))
            nc.sync.dma_start(bqs[:], bqs_d[:])
            nc.vector.memset(va[:, :, :, DH:DH + 1], 1.0)

            xT_r = xT_d.rearrange("(do dp) s -> dp do s", dp=128)

            def qkv_steps(sb, xblk, steps):
                # one thunk per matmul; PSUM->SBUF copy rides on the last
                for wt, dst, bias in ((wq, qt, bqs), (wk, kt, None)):
                    for ft in range(4):
                        st = {}
                        for do in range(8):
                            def th(wt=wt, dst=dst, bias=bias, ft=ft, do=do,
                                   st=st, sb=sb, xblk=xblk):
                                if do == 0:
                                    st['ps'] = ps_o.tile([128, 512], F32, name="psc",
                                                         space="PSUM", tag="mm")
                                nc.tensor.matmul(
                                    st['ps'][:], wt[:, do, ft * 128:(ft + 1) * 128],
                                    xblk[:, do, :], start=(do == 0), stop=(do == 7))
                                if do == 7:
                                    d = dst[:, ft, sb * 512:(sb + 1) * 512]
                                    if bias is not None:
                                        nc.vector.tensor_scalar_add(
                                            d, st['ps'][:], bias[:, ft:ft + 1])
                                    else:
                                        nc.vector.tensor_copy(d, st['ps'][:])
                            steps.append(th)
                for stt in range(4):
                    st = {}
                    for do in range(8):
                        def th(stt=stt, do=do, st=st, sb=sb, xblk=xblk):
                            if do == 0:
                                st['ps'] = ps_o.tile([128, 512], F32, name="psv",
                                                     space="PSUM", tag="mm")
                            nc.tensor.matmul(
                                st['ps'][:], xblk[:, do, stt * 128:(stt + 1) * 128],
                                wv[:, do, :], start=(do == 0), stop=(do == 7))
                            if do == 7:
                                nc.vector.tensor_copy(
                                    va[:, sb * 4 + stt, :, :DH],
                                    st['ps'][:].rearrange("p (h d) -> p h d", h=8))
                        steps.append(th)

            def proj_steps(bb, jts, steps):
                for jt in jts:
                    st = {}
                    for co in range(4):
                        def th(jt=jt, co=co, st=st, bb=bb):
                            if co == 0:
                                st['ps'] = ps_o.tile([128, 512], F32, name="psj",
                                                     space="PSUM", tag="mm")
                            nc.tensor.matmul(
                                st['ps'][:], wp[:, co, jt * 128:(jt + 1) * 128],
                                on_[:, co, bb * 512:(bb + 1) * 512],
                                start=(co == 0), stop=(co == 3))
                            if co == 3:
                                osb = op.tile([128, 512], F32, tag="o")
                                nc.vector.tensor_copy(osb[:], st['ps'][:])
                                nc.sync.dma_start(
                                    out_d[jt * 128:(jt + 1) * 128,
                                          bb * 512:(bb + 1) * 512], osb[:])
                        steps.append(th)

            # ---- QKV for s-block 0 runs upfront (nothing to overlap yet) ----
            xblk0 = xp.tile([128, 8, 512], BF16, tag="x")
            nc.gpsimd.dma_start(xblk0[:], xT_r[:, :, 0:512])
            nc.gpsimd.dma_start(msk[:], msk_d[:])
            nc.gpsimd.dma_start(wv[:], wvT_d.rearrange("(do dp) f -> dp do f", dp=128))
            nc.gpsimd.dma_start(wp[:], wpT_d.rearrange("(co cp) j -> cp co j", cp=128))
            s0 = []
            qkv_steps(0, xblk0, s0)
            for th in s0:
                th()

            for b in range(4):
                steps = []
                if b >= 1:
                    proj_steps(b - 1, range(8), steps)
                if b < 3:
                    xblkn = xp.tile([128, 8, 512], BF16, tag="x")
                    nc.gpsimd.dma_start(
                        xblkn[:], xT_r[:, :, (b + 1) * 512:(b + 2) * 512])
                    qkv_steps(b + 1, xblkn, steps)
                it = iter(steps)

                nt = 4 * b + 4
                for p in range(4):  # head pairs (2p, 2p+1)
                    ot0 = ps_o.tile([DH + 1, 512], F32, space="PSUM", tag="ot")
                    ot1 = ps_o.tile([DH + 1, 512], F32, space="PSUM", tag="ot")
                    for t in range(nt):
                        m = t - 4 * b          # >= 0 on diagonal blocks
                        c0 = 128 * m if m > 0 else 0
                        ksl = slice(t * 128, (t + 1) * 128)
                        qsl = slice(b * 512 + c0, (b + 1) * 512)
                        spr = ps_s.tile([128, 2, 512], F32, space="PSUM", tag="s")
                        nc.tensor.matmul(spr[:, 0, c0:], kt[0:64, p, ksl],
                                         qt[0:64, p, qsl], start=True, stop=True)
                        nc.tensor.matmul(spr[:, 1, c0:], kt[64:128, p, ksl],
                                         qt[64:128, p, qsl], start=True, stop=True)
                        e = ep.tile([128, 2, 512], BF16, tag="e")
                        nc.scalar.activation(e[:, :, c0:], spr[:, :, c0:], AF.Exp)
                        if m >= 0:  # diagonal block: causal mask on the 128-strip
                            nc.vector.tensor_tensor(e[:, :, c0:c0 + 128],
                                                    e[:, :, c0:c0 + 128],
                                                    msk[:], MUL)
                        # one PE filler matmul absorbs the head-of-line wait on
                        # exp; two would push PE past the ACT pace (1147ns/t)
                        th = next(it, None)
                        if th is not None:
                            th()
                        nc.tensor.matmul(ot0[:, c0:], va[:, t, 2 * p, :],
                                         e[:, 0, c0:],
                                         start=(t == 0), stop=(t == nt - 1),
                                         skip_group_check=True)
                        nc.tensor.matmul(ot1[:, c0:], va[:, t, 2 * p + 1, :],
                                         e[:, 1, c0:],
                                         start=(t == 0), stop=(t == nt - 1),
                                         skip_group_check=True)
                    for h, otp in ((2 * p, ot0), (2 * p + 1, ot1)):
                        den = sp.tile([1, 512], F32, tag="den")
                        nc.vector.tensor_copy(den[:], otp[DH:DH + 1, :])
                        rec = sp.tile([1, 512], F32, tag="rec")
                        nc.vector.reciprocal_approx_fast(rec[:], den[:])
                        rb = sp.tile([DH, 512], F32, tag="rb")
                        nc.gpsimd.partition_broadcast(rb[:], rec[:])
                        r0 = 64 * (h % 2)
                        dst = on_[r0:r0 + 64, p, b * 512:(b + 1) * 512]
                        nc.vector.tensor_tensor(dst, otp[0:DH, :], rb[:], MUL)
                for th in it:  # drain leftover filler
                    th()
            s3 = []
            proj_steps(3, range(8), s3)
            for th in s3:
                th()

    nc.compile()
    return nc


def kernel(x, Wq, bq, Wk, bk, Wv, bv, Wp, bp):
    global _NC
    if _NC is None:
        _NC = _build()

    x = np.asarray(x, np.float32)
    Wq, bq = np.asarray(Wq, np.float32), np.asarray(bq, np.float32)
    Wk, bk = np.asarray(Wk, np.float32), np.asarray(bk, np.float32)
    Wv, bv = np.asarray(Wv, np.float32), np.asarray(bv, np.float32)
    Wp, bp = np.asarray(Wp, np.float32), np.asarray(bp, np.float32)

    # diagonal-strip causal mask, duplicated for the two heads of a pair
    i = np.arange(128)[:, None]
    j = np.arange(128)[None, :]
    mstrip = (i <= j).astype(np.float32)            # [128, 128]
    msk = np.broadcast_to(mstrip[:, None, :], (128, 2, 128)).copy()

    # host-folded constant: attn rows sum to 1 -> out += bv, then @Wp.T
    host_bias = Wp @ bv + bp                        # [D]

    in_maps = []
    for c in range(N_CORES):
        b, g = c // 2, c % 2
        hs = range(8 * g, 8 * g + 8)
        kidx = np.array([dh * 16 + h for h in hs for dh in range(DH)])
        fsl = slice(FG * g, FG * (g + 1))
        in_maps.append({
            "xT": np.ascontiguousarray(x[b].T),
            "wqT": np.ascontiguousarray((SCALE * Wq[fsl]).T),
            "wkT": np.ascontiguousarray(Wk[kidx].T),
            "wvT": np.ascontiguousarray(Wv[fsl].T),
            "wpT": np.ascontiguousarray(Wp[:, fsl].T),
            "bqs": np.ascontiguousarray((SCALE * bq[fsl]).reshape(4, 128).T),
            "msk": msk,
        })

    res = run_bass_kernel_spmd(_NC, in_maps, core_ids=list(range(N_CORES)))
    out = np.empty((B, S, D), np.float32)
    for b in range(B):
        acc = res.results[2 * b]["outT"] + res.results[2 * b + 1]["outT"]
        out[b] = acc.T + host_bias
    return out
